# revision 26
# baseline (speedup 1.0000x reference)
"""Trainium2 Bass kernel for MultiHeadAttention (B=2, S=2048, D=1024, H=16).

Sharding: 8 cores = 2 batches x 4 head-groups (4 heads each).
Each core computes QKV projections for its (batch, 4 heads), full causal
attention over seq, and a partial output projection (its 256 rows of Wo).
Host sums the 4 partials per batch and adds bo_eff = bo + bv @ Wo
(the V bias passes through softmax rows which sum to 1).

Compute structure per core (all matmuls in float32r: single-pass fp32,
~4x faster than float32's LOW_HIGH two-pass mode):
  - x^T (pre-transposed on host) -> QT/KT [d, S] and V [S, d] projections
  - orientation A: logits[q, k] per q-tile -> causal mask -> exp (ACT,
    fused row-sum) -> normalize (GpSimd) -> DMA attn rows (+ zero tails)
  - orientation B: logits[k, q] per k-tile -> mask -> exp -> attn@V with
    V stationary, accumulated over k-tiles into persistent PSUM [2 heads, S]
    (unnormalized; B needs no transposes and no row sums)
  - output projection per head (K=64), normalization folded into the
    PSUM->SBUF accumulation: out += psum_h * inv_h[q] (per-partition scalar)

Returns (out, attn) matching reference.py.
"""

import sys

sys.path.insert(0, "/opt/trn_rl_repo")

from contextlib import ExitStack

import numpy as np

import concourse.bacc as bacc
import concourse.bass as bass
import concourse.mybir as mybir
import concourse.tile as tile
from concourse.bass_utils import run_bass_kernel_spmd
from concourse.masks import make_causal_mask

B, S, D, H = 2, 2048, 1024, 16
DEPTH = D // H  # 64
HG = 4  # head-groups (cores per batch)
HPG = H // HG  # heads per core = 4
DG = HPG * DEPTH  # 256 d-columns per core
P = 128
NKT = D // P  # 8 contraction tiles for projections
NQT = S // P  # 16 query tiles
SCALE = 1.0 / float(np.sqrt(DEPTH))  # 0.125
NEG = -1.0e17
F32 = mybir.dt.float32
F32R = mybir.dt.float32r
EXP = mybir.ActivationFunctionType.Exp

_nc_cache = {}


def _build_nc():
    if "nc" in _nc_cache:
        return _nc_cache["nc"]
    nc = bacc.Bacc()

    xq_t = nc.declare_dram_parameter("xq_t", [D, S], F32R, isOutput=False)
    xk_t = nc.declare_dram_parameter("xk_t", [D, S], F32R, isOutput=False)
    xv_t = nc.declare_dram_parameter("xv_t", [D, S], F32R, isOutput=False)
    wq = nc.declare_dram_parameter("wq", [D, DG], F32R, isOutput=False)
    wk = nc.declare_dram_parameter("wk", [D, DG], F32R, isOutput=False)
    wv = nc.declare_dram_parameter("wv", [D, DG], F32R, isOutput=False)
    wo = nc.declare_dram_parameter("wo", [DG, D], F32R, isOutput=False)
    bq = nc.declare_dram_parameter("bq", [DG], F32, isOutput=False)
    bk = nc.declare_dram_parameter("bk", [DG], F32, isOutput=False)

    attn_out = nc.declare_dram_parameter("attn_out", [HPG, S, S], F32, isOutput=True)
    out_part = nc.declare_dram_parameter("out_part", [S, D], F32, isOutput=True)

    _emit(nc, xq_t, xk_t, xv_t, wq, wk, wv, wo, bq, bk, attn_out, out_part)
    nc.compile()
    _nc_cache["nc"] = nc
    return nc


def _emit(nc, xq_t, xk_t, xv_t, wq, wk, wv, wo, bq, bk, attn_out, out_part):
    with tile.TileContext(nc) as tc, ExitStack() as ctx:
        singles = ctx.enter_context(tc.tile_pool(name="singles", bufs=1))
        ps = ctx.enter_context(tc.tile_pool(name="ps", bufs=2, space="PSUM"))
        opsT = ctx.enter_context(tc.tile_pool(name="opsT", bufs=1, space="PSUM"))
        attn_pool = ctx.enter_context(tc.tile_pool(name="attn", bufs=3))
        expT_pool = ctx.enter_context(tc.tile_pool(name="expT", bufs=3))
        outp_pool = ctx.enter_context(tc.tile_pool(name="outp", bufs=2))
        stats = ctx.enter_context(tc.tile_pool(name="stats", bufs=6))

        # --- constants -------------------------------------------------
        # cmask[q, k] (A orientation): 0 where q >= k else -1e17
        cmask = singles.tile([P, P], F32)
        make_causal_mask(nc, cmask, mask_val=NEG)
        # cmaskT[k, q] (B orientation): 0 where q >= k else -1e17
        cmaskT = singles.tile([P, P], F32)
        nc.gpsimd.memset(cmaskT, 0.0)
        nc.gpsimd.affine_select(
            out=cmaskT,
            in_=cmaskT,
            compare_op=mybir.AluOpType.is_ge,
            fill=NEG,
            base=0,
            pattern=[[1, P]],  # iota = -k + q >= 0 ? keep : fill
            channel_multiplier=-1,
        )
        zero_t = singles.tile([P, S - P], F32)
        nc.vector.memset(zero_t, 0.0)
        zbias = singles.tile([P, 1], F32)
        nc.vector.memset(zbias, 0.0)

        # --- persistent activations / weights --------------------------
        bq_sb = singles.tile([P, 2], F32)
        bk_sb = singles.tile([P, 2], F32)
        nc.sync.dma_start(out=bq_sb, in_=bq.rearrange("(c p) -> p c", p=P))
        nc.sync.dma_start(out=bk_sb, in_=bk.rearrange("(c p) -> p c", p=P))

        QT = singles.tile([P, 2, S], F32R)  # [p, c, s]; head h at chunk h//2,
        KT = singles.tile([P, 2, S], F32R)  # partitions (h%2)*64 ..
        V_sb = singles.tile([P, NQT, DG], F32R)  # [p, st, m]

        # --- projections ------------------------------------------------
        with tc.tile_pool(name="wpool", bufs=1) as wpool, tc.tile_pool(
            name="xfull", bufs=1
        ) as xfull:
            wv_sb = wpool.tile([P, NKT, DG], F32R, tag="wv")
            nc.sync.dma_start(out=wv_sb, in_=wv.rearrange("(kt p) m -> p kt m", p=P))
            xv_sb = xfull.tile([P, NKT, S], F32R, tag="x")
            nc.sync.dma_start(out=xv_sb, in_=xv_t.rearrange("(kt p) s -> p kt s", p=P))
            for st in range(NQT):
                pv = ps.tile([P, DG], F32, tag="ps")
                for kt in range(NKT):
                    nc.tensor.matmul(
                        pv,
                        lhsT=xv_sb[:, kt, st * P : (st + 1) * P],
                        rhs=wv_sb[:, kt, :],
                        start=(kt == 0),
                        stop=(kt == NKT - 1),
                    )
                nc.vector.tensor_copy(V_sb[:, st, :], pv)

            for x_dram, w_dram, b_sb, dst, wtag in (
                (xq_t, wq, bq_sb, QT, "wq"),
                (xk_t, wk, bk_sb, KT, "wk"),
            ):
                w_sb = wpool.tile([P, NKT, DG], F32R, tag="wv")
                nc.sync.dma_start(
                    out=w_sb, in_=w_dram.rearrange("(kt p) m -> p kt m", p=P)
                )
                x_sb = xfull.tile([P, NKT, S], F32R, tag="x")
                nc.sync.dma_start(
                    out=x_sb, in_=x_dram.rearrange("(kt p) s -> p kt s", p=P)
                )
                for c in range(2):
                    for sq in range(4):
                        pq = ps.tile([P, 512], F32, tag="ps")
                        for kt in range(NKT):
                            nc.tensor.matmul(
                                pq,
                                lhsT=w_sb[:, kt, c * P : (c + 1) * P],
                                rhs=x_sb[:, kt, sq * 512 : (sq + 1) * 512],
                                start=(kt == 0),
                                stop=(kt == NKT - 1),
                            )
                        nc.vector.tensor_scalar_add(
                            dst[:, c, sq * 512 : (sq + 1) * 512], pq, b_sb[:, c : c + 1]
                        )

        # --- attention --------------------------------------------------
        # (separate pool, opened after the projection pools close, so the
        # SBUF allocator can reuse the x/w space)
        post = ctx.enter_context(tc.tile_pool(name="post", bufs=1))
        wo_sb = post.tile([64, HPG, D], F32R)
        nc.sync.dma_start(out=wo_sb, in_=wo.rearrange("(h p) n -> p h n", p=64))
        ocT = post.tile([64, HPG, S], F32R)  # [p, h, q]: unnormalized out^T
        invh = [
            post.tile([P, NQT], F32, tag=f"invh{h}", name=f"invh{h}")
            for h in range(HPG)
        ]
        for h in range(HPG):
            o_psT = [
                opsT.tile([P, 512], F32, tag=f"opsT{g}", name=f"opsT{g}")
                for g in range(4)
            ]
            if True:
                c = h // 2
                pb = (h % 2) * 64
                qh = QT[pb : pb + 64, c, :]
                kh = KT[pb : pb + 64, c, :]

                # ---- interleaved orientation A (attn rows out) and
                # orientation B (attn @ V, unnormalized). A and B are fully
                # independent, so interleaving keeps more work in flight and
                # the PE stream dense. attn@V for k-tile kt is emitted one
                # iteration later so the in-order PE never waits on exp_B.
                def emit_attn_v(kt, expT):
                    s0 = ((kt * P) // 512) * 512
                    for g in range(s0 // 512, 4):
                        nc.tensor.matmul(
                            o_psT[g][0:64, :],
                            lhsT=V_sb[:, kt, h * DEPTH : (h + 1) * DEPTH],
                            rhs=expT[:, g * 512 : (g + 1) * 512],
                            start=(kt == 0),
                            stop=(kt == min(NQT - 1, 4 * g + 3)),
                        )

                prev = None
                for i in range(NQT):
                    # -- A iteration (qt = i) --
                    qt = i
                    row = (qt + 1) * P
                    nparts = (row + 511) // 512
                    attn_sb = attn_pool.tile([P, S], F32, tag="attn")
                    sums = stats.tile([P, 4], F32, tag="sums")
                    for ip in range(nparts):
                        p0 = ip * 512
                        w = min(512, row - p0)
                        lp = ps.tile([P, 512], F32, tag="ps")
                        nc.tensor.matmul(
                            lp[:, :w],
                            lhsT=qh[:, qt * P : (qt + 1) * P],
                            rhs=kh[:, p0 : p0 + w],
                            start=True,
                            stop=True,
                        )
                        if p0 <= qt * P < p0 + w:  # diagonal block in this part
                            d0 = qt * P - p0
                            nc.vector.tensor_add(
                                lp[:, d0 : d0 + P], lp[:, d0 : d0 + P], cmask
                            )
                        nc.scalar.activation(
                            out=attn_sb[:, p0 : p0 + w],
                            in_=lp[:, :w],
                            func=EXP,
                            bias=zbias,
                            scale=SCALE,
                            accum_out=sums[:, ip : ip + 1],
                        )
                    inv = stats.tile([P, 1], F32, tag="inv")
                    s1 = stats.tile([P, 1], F32, tag="s1")
                    nc.vector.reduce_sum(
                        s1, sums[:, :nparts], axis=mybir.AxisListType.X
                    )
                    nc.vector.reciprocal(inv, s1)
                    nc.vector.tensor_copy(invh[h][:, qt : qt + 1], inv)
                    nc.vector.tensor_scalar_mul(
                        attn_sb[:, :row], attn_sb[:, :row], inv
                    )
                    nc.sync.dma_start(
                        out=attn_out[h, qt * P : (qt + 1) * P, 0:row],
                        in_=attn_sb[:, :row],
                    )
                    if row < S:
                        nc.sync.dma_start(
                            out=attn_out[h, qt * P : (qt + 1) * P, row:S],
                            in_=zero_t[:, : S - row],
                        )

                    # -- B iteration (kt = i) --
                    kt = i
                    k0 = kt * P
                    s0 = (k0 // 512) * 512
                    expT = expT_pool.tile([P, S], F32R, tag="expT")
                    for cs in range(s0, S, 512):
                        lb = ps.tile([P, 512], F32, tag="ps")
                        nc.tensor.matmul(
                            lb,
                            lhsT=kh[:, k0 : k0 + P],
                            rhs=qh[:, cs : cs + 512],
                            start=True,
                            stop=True,
                        )
                        if cs <= k0 < cs + 512:  # diagonal block
                            d0 = k0 - cs
                            nc.vector.tensor_add(
                                lb[:, d0 : d0 + P], lb[:, d0 : d0 + P], cmaskT
                            )
                        nc.scalar.activation(
                            out=expT[:, cs : cs + 512],
                            in_=lb,
                            func=EXP,
                            bias=zbias,
                            scale=SCALE,
                        )
                    if s0 < k0:  # zero the sub-causal garbage columns
                        nc.vector.tensor_scalar_mul(
                            expT[:, s0:k0], expT[:, s0:k0], 0.0
                        )
                    if prev is not None:
                        emit_attn_v(*prev)
                    prev = (kt, expT)
                emit_attn_v(*prev)
            for g in range(4):
                nc.vector.tensor_copy(
                    ocT[:, h, g * 512 : (g + 1) * 512], o_psT[g][0:64, :]
                )

        # --- output projection (per head, normalization fused) ----------
        for qt in range(NQT):
            outp = outp_pool.tile([P, D], F32, tag="outp")
            for h in range(HPG):
                iv = invh[h][:, qt : qt + 1]
                for ncn in range(2):
                    po = ps.tile([P, 512], F32, tag="ps")
                    nc.tensor.matmul(
                        po,
                        lhsT=ocT[:, h, qt * P : (qt + 1) * P],
                        rhs=wo_sb[:, h, ncn * 512 : (ncn + 1) * 512],
                        start=True,
                        stop=True,
                    )
                    dst = outp[:, ncn * 512 : (ncn + 1) * 512]
                    if h == 0:
                        nc.vector.tensor_scalar_mul(dst, po, iv)
                    else:
                        nc.vector.scalar_tensor_tensor(
                            out=dst,
                            in0=po,
                            scalar=iv,
                            in1=dst,
                            op0=mybir.AluOpType.mult,
                            op1=mybir.AluOpType.add,
                        )
            nc.sync.dma_start(out=out_part[qt * P : (qt + 1) * P, :], in_=outp)


def kernel(**inputs):
    v = np.asarray(inputs["v"], np.float32)
    k = np.asarray(inputs["k"], np.float32)
    q = np.asarray(inputs["q"], np.float32)
    Wq = np.asarray(inputs["Wq"], np.float32)
    Wk = np.asarray(inputs["Wk"], np.float32)
    Wv = np.asarray(inputs["Wv"], np.float32)
    Wo = np.asarray(inputs["Wo"], np.float32)
    bq = np.asarray(inputs["bq"], np.float32)
    bk = np.asarray(inputs["bk"], np.float32)
    bv = np.asarray(inputs["bv"], np.float32)
    bo = np.asarray(inputs["bo"], np.float32)

    nc = _build_nc()

    xT = {}
    for b in range(B):
        xT[("q", b)] = np.ascontiguousarray(q[b].T)
        xT[("k", b)] = np.ascontiguousarray(k[b].T)
        xT[("v", b)] = np.ascontiguousarray(v[b].T)

    in_maps = []
    for core in range(8):
        b, hg = core // HG, core % HG
        cols = slice(hg * DG, (hg + 1) * DG)
        in_maps.append(
            {
                "xq_t": xT[("q", b)],
                "xk_t": xT[("k", b)],
                "xv_t": xT[("v", b)],
                "wq": np.ascontiguousarray(Wq[:, cols]),
                "wk": np.ascontiguousarray(Wk[:, cols]),
                "wv": np.ascontiguousarray(Wv[:, cols]),
                "wo": np.ascontiguousarray(Wo[cols, :]),
                "bq": np.ascontiguousarray(bq[cols]),
                "bk": np.ascontiguousarray(bk[cols]),
            }
        )

    res = run_bass_kernel_spmd(nc, in_maps, list(range(8))).results

    bo_eff = bo + bv @ Wo
    out = np.zeros((B, S, D), np.float32)
    attn = np.empty((B, H, S, S), np.float32)
    for core in range(8):
        b, hg = core // HG, core % HG
        attn[b, hg * HPG : (hg + 1) * HPG] = res[core]["attn_out"]
        out[b] += res[core]["out_part"]
    out += bo_eff[None, None, :]
    return out, attn


# revision 29
# speedup vs baseline: 1.2810x; 1.2810x over previous
"""Trainium2 Bass kernel for MultiHeadAttention (B=2, S=2048, D=1024, H=16).

Sharding: 8 cores = 2 batches x 4 head-groups (4 heads each).
Each core computes QKV projections for its (batch, 4 heads), full causal
attention over seq, and a partial output projection (its 256 rows of Wo).
Host sums the 4 partials per batch and adds bo_eff = bo + bv @ Wo
(the V bias passes through softmax rows which sum to 1).

Compute structure per core (all matmuls in float32r: single-pass fp32,
~4x faster than float32's LOW_HIGH two-pass mode):
  - x^T (pre-transposed on host) -> QT/KT [d, S] and V [S, d] projections
  - heads processed in pairs at PE row offsets 0/64 (K=64 matmuls of the
    two heads overlap in the systolic array)
  - orientation A: logits[q, k] per q-tile -> causal mask -> exp (ACT,
    fused row-sum) -> normalize (DVE) -> DMA attn rows (+ zero tails)
  - orientation B: logits[k, q] per k-tile -> mask -> exp -> attn@V with
    V stationary in 2-k-tile PSUM chains, drained into SBUF accumulators
    (unnormalized; B needs no transposes and no row sums)
  - output projection per head (K=64), normalization folded into the
    PSUM->SBUF accumulation: out += psum_h * inv_h[q] (per-partition scalar)

Returns (out, attn) matching reference.py.
"""

import sys

sys.path.insert(0, "/opt/trn_rl_repo")

from contextlib import ExitStack

import numpy as np

import concourse.bacc as bacc
import concourse.bass as bass
import concourse.mybir as mybir
import concourse.tile as tile
from concourse.bass_utils import run_bass_kernel_spmd
from concourse.masks import make_causal_mask

B, S, D, H = 2, 2048, 1024, 16
DEPTH = D // H  # 64
HG = 4  # head-groups (cores per batch)
HPG = H // HG  # heads per core = 4
DG = HPG * DEPTH  # 256 d-columns per core
P = 128
NKT = D // P  # 8 contraction tiles for projections
NQT = S // P  # 16 query tiles
SCALE = 1.0 / float(np.sqrt(DEPTH))  # 0.125
NEG = -1.0e17
F32 = mybir.dt.float32
F32R = mybir.dt.float32r
EXP = mybir.ActivationFunctionType.Exp

_nc_cache = {}


def _build_nc():
    if "nc" in _nc_cache:
        return _nc_cache["nc"]
    nc = bacc.Bacc()

    xq_t = nc.declare_dram_parameter("xq_t", [D, S], F32R, isOutput=False)
    xk_t = nc.declare_dram_parameter("xk_t", [D, S], F32R, isOutput=False)
    xv_t = nc.declare_dram_parameter("xv_t", [D, S], F32R, isOutput=False)
    wq = nc.declare_dram_parameter("wq", [D, DG], F32R, isOutput=False)
    wk = nc.declare_dram_parameter("wk", [D, DG], F32R, isOutput=False)
    wv = nc.declare_dram_parameter("wv", [D, DG], F32R, isOutput=False)
    wo = nc.declare_dram_parameter("wo", [DG, D], F32R, isOutput=False)
    bq = nc.declare_dram_parameter("bq", [DG], F32, isOutput=False)
    bk = nc.declare_dram_parameter("bk", [DG], F32, isOutput=False)

    attn_out = nc.declare_dram_parameter("attn_out", [HPG, S, S], F32, isOutput=True)
    out_part = nc.declare_dram_parameter("out_part", [S, D], F32, isOutput=True)

    _emit(nc, xq_t, xk_t, xv_t, wq, wk, wv, wo, bq, bk, attn_out, out_part)
    nc.compile()
    _nc_cache["nc"] = nc
    return nc


def _emit(nc, xq_t, xk_t, xv_t, wq, wk, wv, wo, bq, bk, attn_out, out_part):
    with tile.TileContext(nc) as tc, ExitStack() as ctx:
        singles = ctx.enter_context(tc.tile_pool(name="singles", bufs=1))
        ps = ctx.enter_context(tc.tile_pool(name="ps", bufs=8, space="PSUM"))

        # --- constants -------------------------------------------------
        # cmask[q, k] (A orientation): 0 where q >= k else -1e17
        cmask = singles.tile([P, P], F32)
        make_causal_mask(nc, cmask, mask_val=NEG)
        # cmaskT[k, q] (B orientation): 0 where q >= k else -1e17
        cmaskT = singles.tile([P, P], F32)
        nc.gpsimd.memset(cmaskT, 0.0)
        nc.gpsimd.affine_select(
            out=cmaskT,
            in_=cmaskT,
            compare_op=mybir.AluOpType.is_ge,
            fill=NEG,
            base=0,
            pattern=[[1, P]],  # iota = -k + q >= 0 ? keep : fill
            channel_multiplier=-1,
        )
        zero_t = singles.tile([P, 1024], F32)
        nc.vector.memset(zero_t, 0.0)
        zbias = singles.tile([P, 1], F32)
        nc.vector.memset(zbias, 0.0)

        # --- persistent activations / weights --------------------------
        bq_sb = singles.tile([P, 2], F32)
        bk_sb = singles.tile([P, 2], F32)
        nc.sync.dma_start(out=bq_sb, in_=bq.rearrange("(c p) -> p c", p=P))
        nc.sync.dma_start(out=bk_sb, in_=bk.rearrange("(c p) -> p c", p=P))

        QT = singles.tile([P, 2, S], F32R)  # [p, c, s]; head h at chunk h//2,
        KT = singles.tile([P, 2, S], F32R)  # partitions (h%2)*64 ..
        V_sb = singles.tile([P, NQT, DG], F32R)  # [p, st, m]

        # --- projections ------------------------------------------------
        with tc.tile_pool(name="wpool", bufs=1) as wpool, tc.tile_pool(
            name="xfull", bufs=1
        ) as xfull:
            wv_sb = wpool.tile([P, NKT, DG], F32R, tag="wv")
            nc.sync.dma_start(out=wv_sb, in_=wv.rearrange("(kt p) m -> p kt m", p=P))
            xv_sb = xfull.tile([P, NKT, S], F32R, tag="x")
            nc.sync.dma_start(out=xv_sb, in_=xv_t.rearrange("(kt p) s -> p kt s", p=P))
            for st in range(NQT):
                pv = ps.tile([P, DG], F32, tag="ps")
                for kt in range(NKT):
                    nc.tensor.matmul(
                        pv,
                        lhsT=xv_sb[:, kt, st * P : (st + 1) * P],
                        rhs=wv_sb[:, kt, :],
                        start=(kt == 0),
                        stop=(kt == NKT - 1),
                    )
                nc.vector.tensor_copy(V_sb[:, st, :], pv)

            for x_dram, w_dram, b_sb, dst in (
                (xq_t, wq, bq_sb, QT),
                (xk_t, wk, bk_sb, KT),
            ):
                w_sb = wpool.tile([P, NKT, DG], F32R, tag="wv")
                nc.sync.dma_start(
                    out=w_sb, in_=w_dram.rearrange("(kt p) m -> p kt m", p=P)
                )
                x_sb = xfull.tile([P, NKT, S], F32R, tag="x")
                nc.sync.dma_start(
                    out=x_sb, in_=x_dram.rearrange("(kt p) s -> p kt s", p=P)
                )
                for c in range(2):
                    for sq in range(4):
                        pq = ps.tile([P, 512], F32, tag="ps")
                        for kt in range(NKT):
                            nc.tensor.matmul(
                                pq,
                                lhsT=w_sb[:, kt, c * P : (c + 1) * P],
                                rhs=x_sb[:, kt, sq * 512 : (sq + 1) * 512],
                                start=(kt == 0),
                                stop=(kt == NKT - 1),
                            )
                        nc.vector.tensor_scalar_add(
                            dst[:, c, sq * 512 : (sq + 1) * 512], pq, b_sb[:, c : c + 1]
                        )

        # --- attention --------------------------------------------------
        # (separate pool, opened after the projection pools close, so the
        # SBUF allocator can reuse the x/w space)
        post = ctx.enter_context(tc.tile_pool(name="post", bufs=1))
        attn_pool = ctx.enter_context(tc.tile_pool(name="attn", bufs=4))
        expT_pool = ctx.enter_context(tc.tile_pool(name="expT", bufs=4))
        oacc_pool = ctx.enter_context(tc.tile_pool(name="oacc", bufs=10))
        outp_pool = ctx.enter_context(tc.tile_pool(name="outp", bufs=2))
        stats = ctx.enter_context(tc.tile_pool(name="stats", bufs=6))
        wo_sb = post.tile([64, HPG, D], F32R)
        nc.sync.dma_start(out=wo_sb, in_=wo.rearrange("(h p) n -> p h n", p=64))
        ocT = post.tile([64, HPG, S], F32R)  # [p, h, q]: unnormalized out^T
        invh = [
            post.tile([P, NQT], F32, tag=f"invh{h}", name=f"invh{h}")
            for h in range(HPG)
        ]

        for hp in range(2):
            heads = (2 * hp, 2 * hp + 1)
            qhs = (QT[0:64, hp, :], QT[64:128, hp, :])
            khs = (KT[0:64, hp, :], KT[64:128, hp, :])
            accs = {}
            for e in (0, 1):
                for g in range(4):
                    accs[(e, g)] = oacc_pool.tile(
                        [64, 512], F32, tag="oacc", name=f"oacc{e}_{g}"
                    )

            def emit_attn_v(j, expT_pair):
                # group kts (2j, 2j+1) contribute to column groups g >= j//2
                for e in (0, 1):
                    hh = heads[e]
                    for g in range(j // 2, 4):
                        o = ps.tile([P, 512], F32, tag="ps", name=f"av{e}g{g}")
                        for kk in (0, 1):
                            kt = 2 * j + kk
                            nc.tensor.matmul(
                                o[0:64, :],
                                lhsT=V_sb[:, kt, hh * DEPTH : (hh + 1) * DEPTH],
                                rhs=expT_pair[kk][e][:, g * 512 : (g + 1) * 512],
                                start=(kk == 0),
                                stop=(kk == 1),
                            )
                        acc = accs[(e, g)]
                        if j == 0:
                            nc.vector.tensor_copy(acc, o[0:64, :])
                        else:
                            nc.vector.tensor_add(acc, acc, o[0:64, :])
                        if j == 2 * g + 1:  # last contributing group
                            nc.vector.tensor_copy(
                                ocT[:, hh, g * 512 : (g + 1) * 512], acc
                            )

            prev_expTs = None
            for i in range(NQT):
                # -- A iteration (qt = i), heads paired --
                row = (i + 1) * P
                nparts = (row + 511) // 512
                attn_sbs = []
                sums_t = []
                for e in (0, 1):
                    attn_sbs.append(
                        attn_pool.tile([P, S], F32, tag="attn", name=f"attn{e}")
                    )
                    sums_t.append(
                        stats.tile([P, 4], F32, tag="sums", name=f"sums{e}")
                    )
                for ip in range(nparts):
                    p0 = ip * 512
                    w = min(512, row - p0)
                    lps = []
                    for e in (0, 1):
                        lp = ps.tile([P, 512], F32, tag="ps", name=f"lpA{e}")
                        lps.append(lp)
                        nc.tensor.matmul(
                            lp[:, :w],
                            lhsT=qhs[e][:, i * P : (i + 1) * P],
                            rhs=khs[e][:, p0 : p0 + w],
                            start=True,
                            stop=True,
                        )
                    for e in (0, 1):
                        if p0 <= i * P < p0 + w:  # diagonal block in this part
                            d0 = i * P - p0
                            nc.vector.tensor_add(
                                lps[e][:, d0 : d0 + P], lps[e][:, d0 : d0 + P], cmask
                            )
                        nc.scalar.activation(
                            out=attn_sbs[e][:, p0 : p0 + w],
                            in_=lps[e][:, :w],
                            func=EXP,
                            bias=zbias,
                            scale=SCALE,
                            accum_out=sums_t[e][:, ip : ip + 1],
                        )
                for e in (0, 1):
                    h = heads[e]
                    inv = stats.tile([P, 1], F32, tag="inv", name=f"inv{e}")
                    s1 = stats.tile([P, 1], F32, tag="s1", name=f"s1{e}")
                    nc.vector.reduce_sum(
                        s1, sums_t[e][:, :nparts], axis=mybir.AxisListType.X
                    )
                    nc.vector.reciprocal(inv, s1)
                    nc.vector.tensor_copy(invh[h][:, i : i + 1], inv)
                    nc.vector.tensor_scalar_mul(
                        attn_sbs[e][:, :row], attn_sbs[e][:, :row], inv
                    )
                    nc.sync.dma_start(
                        out=attn_out[h, i * P : (i + 1) * P, 0:row],
                        in_=attn_sbs[e][:, :row],
                    )
                    tail, toff = S - row, row
                    while tail > 0:
                        w = min(1024, tail)
                        nc.sync.dma_start(
                            out=attn_out[h, i * P : (i + 1) * P, toff : toff + w],
                            in_=zero_t[:, :w],
                        )
                        tail -= w
                        toff += w

                # -- B iteration (kt = i), heads paired --
                k0 = i * P
                s0 = (k0 // 512) * 512
                expTs = []
                for e in (0, 1):
                    expTs.append(
                        expT_pool.tile([P, S], F32R, tag="expT", name=f"expT{e}")
                    )
                for cs in range(s0, S, 512):
                    lbs = []
                    for e in (0, 1):
                        lb = ps.tile([P, 512], F32, tag="ps", name=f"lbB{e}")
                        lbs.append(lb)
                        nc.tensor.matmul(
                            lb,
                            lhsT=khs[e][:, k0 : k0 + P],
                            rhs=qhs[e][:, cs : cs + 512],
                            start=True,
                            stop=True,
                        )
                    for e in (0, 1):
                        if cs <= k0 < cs + 512:  # diagonal block
                            d0 = k0 - cs
                            nc.vector.tensor_add(
                                lbs[e][:, d0 : d0 + P], lbs[e][:, d0 : d0 + P], cmaskT
                            )
                        nc.scalar.activation(
                            out=expTs[e][:, cs : cs + 512],
                            in_=lbs[e],
                            func=EXP,
                            bias=zbias,
                            scale=SCALE,
                        )
                for e in (0, 1):
                    if s0 < k0:  # zero the sub-causal garbage columns
                        nc.vector.tensor_scalar_mul(
                            expTs[e][:, s0:k0], expTs[e][:, s0:k0], 0.0
                        )
                if i % 2 == 1:
                    emit_attn_v(i // 2, (prev_expTs, expTs))
                prev_expTs = expTs

        # --- output projection (per head, normalization fused) ----------
        for qt in range(NQT):
            outp = outp_pool.tile([P, D], F32, tag="outp")
            for h in range(HPG):
                iv = invh[h][:, qt : qt + 1]
                for ncn in range(2):
                    po = ps.tile([P, 512], F32, tag="ps")
                    nc.tensor.matmul(
                        po,
                        lhsT=ocT[:, h, qt * P : (qt + 1) * P],
                        rhs=wo_sb[:, h, ncn * 512 : (ncn + 1) * 512],
                        start=True,
                        stop=True,
                    )
                    dst = outp[:, ncn * 512 : (ncn + 1) * 512]
                    if h == 0:
                        nc.vector.tensor_scalar_mul(dst, po, iv)
                    else:
                        nc.vector.scalar_tensor_tensor(
                            out=dst,
                            in0=po,
                            scalar=iv,
                            in1=dst,
                            op0=mybir.AluOpType.mult,
                            op1=mybir.AluOpType.add,
                        )
            nc.sync.dma_start(out=out_part[qt * P : (qt + 1) * P, :], in_=outp)


def kernel(**inputs):
    v = np.asarray(inputs["v"], np.float32)
    k = np.asarray(inputs["k"], np.float32)
    q = np.asarray(inputs["q"], np.float32)
    Wq = np.asarray(inputs["Wq"], np.float32)
    Wk = np.asarray(inputs["Wk"], np.float32)
    Wv = np.asarray(inputs["Wv"], np.float32)
    Wo = np.asarray(inputs["Wo"], np.float32)
    bq = np.asarray(inputs["bq"], np.float32)
    bk = np.asarray(inputs["bk"], np.float32)
    bv = np.asarray(inputs["bv"], np.float32)
    bo = np.asarray(inputs["bo"], np.float32)

    nc = _build_nc()

    xT = {}
    for b in range(B):
        xT[("q", b)] = np.ascontiguousarray(q[b].T)
        xT[("k", b)] = np.ascontiguousarray(k[b].T)
        xT[("v", b)] = np.ascontiguousarray(v[b].T)

    in_maps = []
    for core in range(8):
        b, hg = core // HG, core % HG
        cols = slice(hg * DG, (hg + 1) * DG)
        in_maps.append(
            {
                "xq_t": xT[("q", b)],
                "xk_t": xT[("k", b)],
                "xv_t": xT[("v", b)],
                "wq": np.ascontiguousarray(Wq[:, cols]),
                "wk": np.ascontiguousarray(Wk[:, cols]),
                "wv": np.ascontiguousarray(Wv[:, cols]),
                "wo": np.ascontiguousarray(Wo[cols, :]),
                "bq": np.ascontiguousarray(bq[cols]),
                "bk": np.ascontiguousarray(bk[cols]),
            }
        )

    res = run_bass_kernel_spmd(nc, in_maps, list(range(8))).results

    bo_eff = bo + bv @ Wo
    out = np.zeros((B, S, D), np.float32)
    attn = np.empty((B, H, S, S), np.float32)
    for core in range(8):
        b, hg = core // HG, core % HG
        attn[b, hg * HPG : (hg + 1) * HPG] = res[core]["attn_out"]
        out[b] += res[core]["out_part"]
    out += bo_eff[None, None, :]
    return out, attn


# revision 30
# speedup vs baseline: 1.4142x; 1.1040x over previous
"""Trainium2 Bass kernel for MultiHeadAttention (B=2, S=2048, D=1024, H=16).

Sharding: 8 cores = 2 batches x 4 head-groups (4 heads each).
Each core computes QKV projections for its (batch, 4 heads), full causal
attention over seq, and a partial output projection (its 256 rows of Wo).
Host sums the 4 partials per batch and adds bo_eff = bo + bv @ Wo
(the V bias passes through softmax rows which sum to 1).

Compute structure per core (all matmuls in float32r: single-pass fp32,
~4x faster than float32's LOW_HIGH two-pass mode):
  - x^T (pre-transposed on host) -> QT/KT [d, S] and V [S, d] projections
  - heads processed in pairs at PE row offsets 0/64 (K=64 matmuls of the
    two heads overlap in the systolic array)
  - orientation A: logits[q, k] per q-tile -> causal mask -> exp (ACT,
    fused row-sum) -> normalize (DVE) -> DMA attn rows (+ zero tails)
  - orientation B: logits[k, q] per k-tile -> mask -> exp -> attn@V with
    V stationary in 2-k-tile PSUM chains, drained into SBUF accumulators
    (unnormalized; B needs no transposes and no row sums)
  - output projection per head (K=64), normalization folded into the
    PSUM->SBUF accumulation: out += psum_h * inv_h[q] (per-partition scalar)

Returns (out, attn) matching reference.py.
"""

import sys

sys.path.insert(0, "/opt/trn_rl_repo")

from contextlib import ExitStack

import numpy as np

import concourse.bacc as bacc
import concourse.bass as bass
import concourse.mybir as mybir
import concourse.tile as tile
from concourse.bass_utils import run_bass_kernel_spmd
from concourse.masks import make_causal_mask

B, S, D, H = 2, 2048, 1024, 16
DEPTH = D // H  # 64
HG = 4  # head-groups (cores per batch)
HPG = H // HG  # heads per core = 4
DG = HPG * DEPTH  # 256 d-columns per core
P = 128
NKT = D // P  # 8 contraction tiles for projections
NQT = S // P  # 16 query tiles
SCALE = 1.0 / float(np.sqrt(DEPTH))  # 0.125
NEG = -1.0e17
F32 = mybir.dt.float32
F32R = mybir.dt.float32r
EXP = mybir.ActivationFunctionType.Exp

_nc_cache = {}


def _build_nc():
    if "nc" in _nc_cache:
        return _nc_cache["nc"]
    nc = bacc.Bacc()

    xq_t = nc.declare_dram_parameter("xq_t", [D, S], F32R, isOutput=False)
    xk_t = nc.declare_dram_parameter("xk_t", [D, S], F32R, isOutput=False)
    xv_t = nc.declare_dram_parameter("xv_t", [D, S], F32R, isOutput=False)
    wq = nc.declare_dram_parameter("wq", [D, DG], F32R, isOutput=False)
    wk = nc.declare_dram_parameter("wk", [D, DG], F32R, isOutput=False)
    wv = nc.declare_dram_parameter("wv", [D, DG], F32R, isOutput=False)
    wo = nc.declare_dram_parameter("wo", [DG, D], F32R, isOutput=False)
    bq = nc.declare_dram_parameter("bq", [DG], F32, isOutput=False)
    bk = nc.declare_dram_parameter("bk", [DG], F32, isOutput=False)

    attn_out = nc.declare_dram_parameter("attn_out", [HPG, S, S], F32, isOutput=True)
    out_part = nc.declare_dram_parameter("out_part", [S, D], F32, isOutput=True)

    _emit(nc, xq_t, xk_t, xv_t, wq, wk, wv, wo, bq, bk, attn_out, out_part)
    nc.compile()
    _nc_cache["nc"] = nc
    return nc


def _emit(nc, xq_t, xk_t, xv_t, wq, wk, wv, wo, bq, bk, attn_out, out_part):
    with tile.TileContext(nc) as tc, ExitStack() as ctx:
        singles = ctx.enter_context(tc.tile_pool(name="singles", bufs=1))
        ps = ctx.enter_context(tc.tile_pool(name="ps", bufs=8, space="PSUM"))

        # --- constants -------------------------------------------------
        # cmask[q, k] (A orientation): 0 where q >= k else -1e17
        cmask = singles.tile([P, P], F32)
        make_causal_mask(nc, cmask, mask_val=NEG)
        # cmaskT[k, q] (B orientation): 0 where q >= k else -1e17
        cmaskT = singles.tile([P, P], F32)
        nc.gpsimd.memset(cmaskT, 0.0)
        nc.gpsimd.affine_select(
            out=cmaskT,
            in_=cmaskT,
            compare_op=mybir.AluOpType.is_ge,
            fill=NEG,
            base=0,
            pattern=[[1, P]],  # iota = -k + q >= 0 ? keep : fill
            channel_multiplier=-1,
        )
        zero_t = singles.tile([P, 1024], F32)
        nc.vector.memset(zero_t, 0.0)
        zbias = singles.tile([P, 1], F32)
        nc.vector.memset(zbias, 0.0)

        # --- persistent activations / weights --------------------------
        bq_sb = singles.tile([P, 2], F32)
        bk_sb = singles.tile([P, 2], F32)
        nc.sync.dma_start(out=bq_sb, in_=bq.rearrange("(c p) -> p c", p=P))
        nc.sync.dma_start(out=bk_sb, in_=bk.rearrange("(c p) -> p c", p=P))

        QT = singles.tile([P, 2, S], F32R)  # [p, c, s]; head h at chunk h//2,
        KT = singles.tile([P, 2, S], F32R)  # partitions (h%2)*64 ..
        V_sb = singles.tile([P, NQT, DG], F32R)  # [p, st, m]

        # --- projections ------------------------------------------------
        with tc.tile_pool(name="wpool", bufs=2) as wpool, tc.tile_pool(
            name="xfull", bufs=2
        ) as xfull:
            wv_sb = wpool.tile([P, NKT, DG], F32R, tag="wv")
            nc.sync.dma_start(out=wv_sb, in_=wv.rearrange("(kt p) m -> p kt m", p=P))
            xv_sb = xfull.tile([P, NKT, S], F32R, tag="x")
            nc.sync.dma_start(out=xv_sb, in_=xv_t.rearrange("(kt p) s -> p kt s", p=P))
            for st in range(NQT):
                pv = ps.tile([P, DG], F32, tag="ps")
                for kt in range(NKT):
                    nc.tensor.matmul(
                        pv,
                        lhsT=xv_sb[:, kt, st * P : (st + 1) * P],
                        rhs=wv_sb[:, kt, :],
                        start=(kt == 0),
                        stop=(kt == NKT - 1),
                    )
                nc.vector.tensor_copy(V_sb[:, st, :], pv)

            for x_dram, w_dram, b_sb, dst in (
                (xq_t, wq, bq_sb, QT),
                (xk_t, wk, bk_sb, KT),
            ):
                w_sb = wpool.tile([P, NKT, DG], F32R, tag="wv")
                nc.sync.dma_start(
                    out=w_sb, in_=w_dram.rearrange("(kt p) m -> p kt m", p=P)
                )
                x_sb = xfull.tile([P, NKT, S], F32R, tag="x")
                nc.sync.dma_start(
                    out=x_sb, in_=x_dram.rearrange("(kt p) s -> p kt s", p=P)
                )
                for c in range(2):
                    for sq in range(4):
                        pq = ps.tile([P, 512], F32, tag="ps")
                        for kt in range(NKT):
                            nc.tensor.matmul(
                                pq,
                                lhsT=w_sb[:, kt, c * P : (c + 1) * P],
                                rhs=x_sb[:, kt, sq * 512 : (sq + 1) * 512],
                                start=(kt == 0),
                                stop=(kt == NKT - 1),
                            )
                        nc.vector.tensor_scalar_add(
                            dst[:, c, sq * 512 : (sq + 1) * 512], pq, b_sb[:, c : c + 1]
                        )

        # --- attention --------------------------------------------------
        # (separate pool, opened after the projection pools close, so the
        # SBUF allocator can reuse the x/w space)
        post = ctx.enter_context(tc.tile_pool(name="post", bufs=1))
        attn_pool = ctx.enter_context(tc.tile_pool(name="attn", bufs=4))
        expT_pool = ctx.enter_context(tc.tile_pool(name="expT", bufs=4))
        oacc_pool = ctx.enter_context(tc.tile_pool(name="oacc", bufs=10))
        outp_pool = ctx.enter_context(tc.tile_pool(name="outp", bufs=2))
        stats = ctx.enter_context(tc.tile_pool(name="stats", bufs=6))
        wo_sb = post.tile([64, HPG, D], F32R)
        nc.sync.dma_start(out=wo_sb, in_=wo.rearrange("(h p) n -> p h n", p=64))
        ocT = post.tile([64, HPG, S], F32R)  # [p, h, q]: unnormalized out^T
        invh = [
            post.tile([P, NQT], F32, tag=f"invh{h}", name=f"invh{h}")
            for h in range(HPG)
        ]

        for hp in range(2):
            heads = (2 * hp, 2 * hp + 1)
            qhs = (QT[0:64, hp, :], QT[64:128, hp, :])
            khs = (KT[0:64, hp, :], KT[64:128, hp, :])
            accs = {}
            for e in (0, 1):
                for g in range(4):
                    accs[(e, g)] = oacc_pool.tile(
                        [64, 512], F32, tag="oacc", name=f"oacc{e}_{g}"
                    )

            def emit_attn_v(j, expT_pair):
                # group kts (2j, 2j+1) contribute to column groups g >= j//2
                for e in (0, 1):
                    hh = heads[e]
                    for g in range(j // 2, 4):
                        o = ps.tile([P, 512], F32, tag="ps", name=f"av{e}g{g}")
                        for kk in (0, 1):
                            kt = 2 * j + kk
                            nc.tensor.matmul(
                                o[0:64, :],
                                lhsT=V_sb[:, kt, hh * DEPTH : (hh + 1) * DEPTH],
                                rhs=expT_pair[kk][e][:, g * 512 : (g + 1) * 512],
                                start=(kk == 0),
                                stop=(kk == 1),
                            )
                        acc = accs[(e, g)]
                        if j == 0:
                            nc.vector.tensor_copy(acc, o[0:64, :])
                        else:
                            nc.vector.tensor_add(acc, acc, o[0:64, :])
                        if j == 2 * g + 1:  # last contributing group
                            nc.vector.tensor_copy(
                                ocT[:, hh, g * 512 : (g + 1) * 512], acc
                            )

            prev_expTs = None
            for i in range(NQT):
                # -- A iteration (qt = i), heads paired --
                row = (i + 1) * P
                nparts = (row + 511) // 512
                attn_sbs = []
                sums_t = []
                for e in (0, 1):
                    attn_sbs.append(
                        attn_pool.tile([P, S], F32, tag="attn", name=f"attn{e}")
                    )
                    sums_t.append(
                        stats.tile([P, 4], F32, tag="sums", name=f"sums{e}")
                    )
                for ip in range(nparts):
                    p0 = ip * 512
                    w = min(512, row - p0)
                    lps = []
                    for e in (0, 1):
                        lp = ps.tile([P, 512], F32, tag="ps", name=f"lpA{e}")
                        lps.append(lp)
                        nc.tensor.matmul(
                            lp[:, :w],
                            lhsT=qhs[e][:, i * P : (i + 1) * P],
                            rhs=khs[e][:, p0 : p0 + w],
                            start=True,
                            stop=True,
                        )
                    for e in (0, 1):
                        if p0 <= i * P < p0 + w:  # diagonal block in this part
                            d0 = i * P - p0
                            nc.vector.tensor_add(
                                lps[e][:, d0 : d0 + P], lps[e][:, d0 : d0 + P], cmask
                            )
                        nc.scalar.activation(
                            out=attn_sbs[e][:, p0 : p0 + w],
                            in_=lps[e][:, :w],
                            func=EXP,
                            bias=zbias,
                            scale=SCALE,
                            accum_out=sums_t[e][:, ip : ip + 1],
                        )
                for e in (0, 1):
                    h = heads[e]
                    inv = stats.tile([P, 1], F32, tag="inv", name=f"inv{e}")
                    s1 = stats.tile([P, 1], F32, tag="s1", name=f"s1{e}")
                    nc.vector.reduce_sum(
                        s1, sums_t[e][:, :nparts], axis=mybir.AxisListType.X
                    )
                    nc.vector.reciprocal(inv, s1)
                    nc.vector.tensor_copy(invh[h][:, i : i + 1], inv)
                    nc.vector.tensor_scalar_mul(
                        attn_sbs[e][:, :row], attn_sbs[e][:, :row], inv
                    )
                    nc.sync.dma_start(
                        out=attn_out[h, i * P : (i + 1) * P, 0:row],
                        in_=attn_sbs[e][:, :row],
                    )
                    tail, toff = S - row, row
                    while tail > 0:
                        w = min(1024, tail)
                        nc.sync.dma_start(
                            out=attn_out[h, i * P : (i + 1) * P, toff : toff + w],
                            in_=zero_t[:, :w],
                        )
                        tail -= w
                        toff += w

                # -- B iteration (kt = i), heads paired --
                k0 = i * P
                s0 = (k0 // 512) * 512
                expTs = []
                for e in (0, 1):
                    expTs.append(
                        expT_pool.tile([P, S], F32R, tag="expT", name=f"expT{e}")
                    )
                for cs in range(s0, S, 512):
                    lbs = []
                    for e in (0, 1):
                        lb = ps.tile([P, 512], F32, tag="ps", name=f"lbB{e}")
                        lbs.append(lb)
                        nc.tensor.matmul(
                            lb,
                            lhsT=khs[e][:, k0 : k0 + P],
                            rhs=qhs[e][:, cs : cs + 512],
                            start=True,
                            stop=True,
                        )
                    for e in (0, 1):
                        if cs <= k0 < cs + 512:  # diagonal block
                            d0 = k0 - cs
                            nc.vector.tensor_add(
                                lbs[e][:, d0 : d0 + P], lbs[e][:, d0 : d0 + P], cmaskT
                            )
                        nc.scalar.activation(
                            out=expTs[e][:, cs : cs + 512],
                            in_=lbs[e],
                            func=EXP,
                            bias=zbias,
                            scale=SCALE,
                        )
                for e in (0, 1):
                    if s0 < k0:  # zero the sub-causal garbage columns
                        nc.vector.tensor_scalar_mul(
                            expTs[e][:, s0:k0], expTs[e][:, s0:k0], 0.0
                        )
                if i % 2 == 1:
                    emit_attn_v(i // 2, (prev_expTs, expTs))
                prev_expTs = expTs

        # --- output projection (per head, normalization fused) ----------
        for qt in range(NQT):
            outp = outp_pool.tile([P, D], F32, tag="outp")
            for h in range(HPG):
                iv = invh[h][:, qt : qt + 1]
                for ncn in range(2):
                    po = ps.tile([P, 512], F32, tag="ps")
                    nc.tensor.matmul(
                        po,
                        lhsT=ocT[:, h, qt * P : (qt + 1) * P],
                        rhs=wo_sb[:, h, ncn * 512 : (ncn + 1) * 512],
                        start=True,
                        stop=True,
                    )
                    dst = outp[:, ncn * 512 : (ncn + 1) * 512]
                    if h == 0:
                        nc.vector.tensor_scalar_mul(dst, po, iv)
                    else:
                        nc.vector.scalar_tensor_tensor(
                            out=dst,
                            in0=po,
                            scalar=iv,
                            in1=dst,
                            op0=mybir.AluOpType.mult,
                            op1=mybir.AluOpType.add,
                        )
            nc.sync.dma_start(out=out_part[qt * P : (qt + 1) * P, :], in_=outp)


def kernel(**inputs):
    v = np.asarray(inputs["v"], np.float32)
    k = np.asarray(inputs["k"], np.float32)
    q = np.asarray(inputs["q"], np.float32)
    Wq = np.asarray(inputs["Wq"], np.float32)
    Wk = np.asarray(inputs["Wk"], np.float32)
    Wv = np.asarray(inputs["Wv"], np.float32)
    Wo = np.asarray(inputs["Wo"], np.float32)
    bq = np.asarray(inputs["bq"], np.float32)
    bk = np.asarray(inputs["bk"], np.float32)
    bv = np.asarray(inputs["bv"], np.float32)
    bo = np.asarray(inputs["bo"], np.float32)

    nc = _build_nc()

    xT = {}
    for b in range(B):
        xT[("q", b)] = np.ascontiguousarray(q[b].T)
        xT[("k", b)] = np.ascontiguousarray(k[b].T)
        xT[("v", b)] = np.ascontiguousarray(v[b].T)

    in_maps = []
    for core in range(8):
        b, hg = core // HG, core % HG
        cols = slice(hg * DG, (hg + 1) * DG)
        in_maps.append(
            {
                "xq_t": xT[("q", b)],
                "xk_t": xT[("k", b)],
                "xv_t": xT[("v", b)],
                "wq": np.ascontiguousarray(Wq[:, cols]),
                "wk": np.ascontiguousarray(Wk[:, cols]),
                "wv": np.ascontiguousarray(Wv[:, cols]),
                "wo": np.ascontiguousarray(Wo[cols, :]),
                "bq": np.ascontiguousarray(bq[cols]),
                "bk": np.ascontiguousarray(bk[cols]),
            }
        )

    res = run_bass_kernel_spmd(nc, in_maps, list(range(8))).results

    bo_eff = bo + bv @ Wo
    out = np.zeros((B, S, D), np.float32)
    attn = np.empty((B, H, S, S), np.float32)
    for core in range(8):
        b, hg = core // HG, core % HG
        attn[b, hg * HPG : (hg + 1) * HPG] = res[core]["attn_out"]
        out[b] += res[core]["out_part"]
    out += bo_eff[None, None, :]
    return out, attn
